# revision 1
# baseline (speedup 1.0000x reference)
"""AttentionMemoryInterface Trainium2 kernel (v3).

Reference computation per batch element b (memory [N=4096, D=128], x [256]):
    mv = x@W_write+b_write; wq = x@W_wq+b_wq; rq = x@W_rq+b_rq
    wl[n] = mem[n,:]@wq ; ww = softmax(wl)
    new_mem = mem*(1-ww) + mv*ww
    rl[n] = new_mem[n,:]@rq ; rw = softmax(rl)
    out = (rw @ new_mem) @ W_ro + b_ro

Algebraic restructure (new_mem never materialized):
    lr[n] = mem[n,:]@rq                  (same pass as wl)
    cbar  = mv@rq                        (scalar per b)
    rl[n] = lr[n] + ww[n]*(cbar - lr[n])
    g[n]  = rw[n]*(1-ww[n]);  s = sum_n rw[n]*ww[n]
    read_out = sum_n g[n]*mem[n,:] + s*mv
    out = read_out @ W_ro + b_ro

v3 structure (per core, 8 batch elements, data-parallel over batch):
  - host pre-transposes memory -> memT [8, 128(d), 4096(n)]; DMA'd flat
    (perfect per-partition-contiguous pattern); no on-chip memory transposes.
  - pass 1: col-tiled matmuls - stationary QP32 [128,32] at 4 column-groups
    of the PE array, moving memT groups [128,512]; 4 concurrent streams fill
    one PSUM bank [128,512] (partitions 32j+q).
  - logit fix-up: 4 PE transposes per bank -> [128(n-sub), (Q,j,q)] layout so
    softmax runs on 128-lane tiles.
  - softmax: ACT exp (with accumulated row sums), DVE elementwise, PE for
    cross-partition sums/broadcasts.
  - g row: SWDGE fold-DMA [128,32] -> [1,4096] in n-order (optionally cast
    f32r); broadcast to [128,512] PSUM quarters via K=1 PE matmuls.
  - pass 2: DVE scalar_tensor_tensor (memT-quarter * g-bcast) with accum_out
    per-partition sums -> read_out; epilogue matmul with bias folded in.
"""

import numpy as np

import concourse.bass as bass
import concourse.bacc as bacc
import concourse.mybir as mybir
import concourse.tile as tile
from concourse.bass_utils import run_bass_kernel_spmd

N_CORES = 8
B, IN_DIM, D, N_SLOTS = 64, 256, 128, 4096
BC = B // N_CORES          # batch per core
NQ = 2 * BC                # 16 query columns (wq x 8 | rq x 8)
NGRP = N_SLOTS // 512      # 8 moving groups per b
F32 = mybir.dt.float32
F32R = mybir.dt.float32r
AX = mybir.AxisListType
ALU = mybir.AluOpType
ACTF = mybir.ActivationFunctionType


def build_nc(loop_n: int = 1, phase: str = "full", bcast: str = "pe_f32r"):
    nc = bacc.Bacc("TRN2", target_bir_lowering=False, debug=False,
                   num_devices=N_CORES)

    x_d = nc.dram_tensor("x", [BC, IN_DIM], F32, kind="ExternalInput")
    memt_d = nc.dram_tensor("memoryT", [BC, D, N_SLOTS], F32,
                            kind="ExternalInput")
    w_wr_d = nc.dram_tensor("W_write", [IN_DIM, D], F32, kind="ExternalInput")
    b_wr_d = nc.dram_tensor("b_write", [1, D], F32, kind="ExternalInput")
    w_wq_d = nc.dram_tensor("W_wq", [IN_DIM, D], F32, kind="ExternalInput")
    b_wq_d = nc.dram_tensor("b_wq", [1, D], F32, kind="ExternalInput")
    w_rq_d = nc.dram_tensor("W_rq", [IN_DIM, D], F32, kind="ExternalInput")
    b_rq_d = nc.dram_tensor("b_rq", [1, D], F32, kind="ExternalInput")
    w_ro_d = nc.dram_tensor("W_ro", [D, IN_DIM], F32, kind="ExternalInput")
    b_ro_d = nc.dram_tensor("b_ro", [1, IN_DIM], F32, kind="ExternalInput")
    ident_d = nc.dram_tensor("ident", [128, 128], F32, kind="ExternalInput")
    onesc_d = nc.dram_tensor("ones_col", [128, 1], F32, kind="ExternalInput")
    onesr_d = nc.dram_tensor("ones_row", [1, 128], F32, kind="ExternalInput")
    out_d = nc.dram_tensor("out", [BC, IN_DIM], F32, kind="ExternalOutput")

    kw = dict(x=x_d.ap(), memt=memt_d.ap(),
              w_wr=w_wr_d.ap(), b_wr=b_wr_d.ap(),
              w_wq=w_wq_d.ap(), b_wq=b_wq_d.ap(),
              w_rq=w_rq_d.ap(), b_rq=b_rq_d.ap(),
              w_ro=w_ro_d.ap(), b_ro=b_ro_d.ap(),
              ident=ident_d.ap(), ones_col=onesc_d.ap(),
              ones_row=onesr_d.ap(), out=out_d.ap(),
              phase=phase, bcast=bcast)
    with tile.TileContext(nc) as tc:
        if loop_n == 1:
            _body(nc, tc, **kw)
        else:
            with tc.For_i(0, loop_n, 1):
                _body(nc, tc, **kw)
    nc.compile()
    return nc


def _body(nc, tc, *, x, memt, w_wr, b_wr, w_wq, b_wq, w_rq, b_rq,
          w_ro, b_ro, ident, ones_col, ones_row, out, phase, bcast):
    from contextlib import ExitStack
    ctx = ExitStack()
    gdt = F32R if bcast == "pe_f32r" else F32
    with ctx:
        consts = ctx.enter_context(tc.tile_pool(name="consts", bufs=1))
        mtp = ctx.enter_context(tc.tile_pool(name="mt", bufs=1))
        ctp = ctx.enter_context(tc.tile_pool(name="ct", bufs=3))
        ltp = ctx.enter_context(tc.tile_pool(name="lt", bufs=3))
        sm = ctx.enter_context(tc.tile_pool(name="sm", bufs=2))
        grp = ctx.enter_context(tc.tile_pool(name="gr", bufs=2))
        ps_ct = ctx.enter_context(tc.tile_pool(name="ps_ct", bufs=2, space="PSUM"))
        ps_fx = ctx.enter_context(tc.tile_pool(name="ps_fx", bufs=1, space="PSUM"))
        ps_gb = ctx.enter_context(tc.tile_pool(name="ps_gb", bufs=2, space="PSUM"))
        ps_sm = ctx.enter_context(tc.tile_pool(name="ps_sm", bufs=2, space="PSUM"))

        # ---------- constants ----------
        ident_sb = consts.tile([128, 128], F32, tag="ident", name="ident_sb")
        nc.sync.dma_start(ident_sb[:], ident)
        onesc_sb = consts.tile([128, 1], F32, tag="onesc", name="onesc_sb")
        nc.sync.dma_start(onesc_sb[:], ones_col)
        onesr_sb = consts.tile([1, 128], F32, tag="onesr", name="onesr_sb")
        nc.sync.dma_start(onesr_sb[:], ones_row)
        onesr_r = consts.tile([1, 128], gdt, tag="onesr_r", name="onesr_r")
        nc.gpsimd.dma_start(onesr_r[:], ones_row)

        w_ro_sb = consts.tile([D, IN_DIM], F32, tag="wro", name="w_ro_sb")
        nc.sync.dma_start(w_ro_sb[:], w_ro)
        b_ro_sb = consts.tile([1, IN_DIM], F32, tag="bro", name="b_ro_sb")
        nc.sync.dma_start(b_ro_sb[:], b_ro)

        proj_w = []
        for name, wd, bd in (("wr", w_wr, b_wr), ("wq", w_wq, b_wq),
                             ("rq", w_rq, b_rq)):
            chunks = []
            for k in range(IN_DIM // 128):
                wt = consts.tile([128, D], F32, tag=f"w_{name}{k}",
                                 name=f"w_{name}{k}")
                nc.sync.dma_start(wt[:], wd[k * 128:(k + 1) * 128, :])
                chunks.append(wt)
            bt = consts.tile([1, D], F32, tag=f"b_{name}", name=f"b_{name}")
            nc.sync.dma_start(bt[:], bd)
            proj_w.append((chunks, bt))

        x_nat = consts.tile([BC, IN_DIM], F32, tag="xnat", name="x_nat")
        nc.sync.dma_start(x_nat[:], x)

        # ---------- memory DMAs (flat, per-partition contiguous) ----------
        m_tiles = []
        for b in range(BC):
            mb = mtp.tile([128, N_SLOTS], F32, tag=f"mem{b}", name=f"memt{b}")
            nc.sync.dma_start(mb[:], memt[b])
            m_tiles.append(mb)

        if phase == "dma":
            dummy = sm.tile([128, BC], F32, tag="dummy", name="dummy")
            for b in range(BC):
                nc.vector.tensor_copy(dummy[:, b:b + 1], m_tiles[b][:, 0:1])
            return

        # ---------- x transpose ----------
        xt = []
        for k in range(IN_DIM // 128):
            ps = ps_sm.tile([128, BC], F32, tag="ps_small", name=f"ps_xt{k}")
            nc.tensor.matmul(ps[:], x_nat[:, k * 128:(k + 1) * 128],
                             ident_sb[0:BC, 0:BC], is_transpose=True)
            t = consts.tile([128, BC], F32, tag=f"xt{k}", name=f"xt{k}")
            nc.scalar.activation(t[:], ps[:], ACTF.Copy)
            xt.append(t)

        # ---------- projections -> mv_t [128, BC], qp32 [128, 32] ----------
        # qp32 columns: [wq (8) | rq (8) | wq (8) | rq (8)] (duplicated to
        # fill all 32 stationary columns of each PE column-group).
        mv_t = consts.tile([128, BC], F32, tag="mvt", name="mv_t")
        qp32 = consts.tile([128, 32], F32, tag="qp32", name="qp32")
        for j, (chunks, bt) in enumerate(proj_w):
            ps = ps_sm.tile([128, BC], F32, tag="ps_small", name=f"ps_proj{j}")
            nc.tensor.matmul(ps[:], bt[:], onesr_sb[:, 0:BC], start=True,
                             stop=False)
            for k in range(IN_DIM // 128):
                nc.tensor.matmul(ps[:], chunks[k][:], xt[k][:],
                                 start=False, stop=(k == IN_DIM // 128 - 1))
            if j == 0:
                nc.scalar.activation(mv_t[:], ps[:], ACTF.Copy)
            else:
                off = (j - 1) * BC
                nc.scalar.activation(qp32[:, off:off + BC], ps[:], ACTF.Copy)
                nc.scalar.activation(qp32[:, 16 + off:16 + off + BC], ps[:],
                                     ACTF.Copy)

        # ---------- cbar ----------
        tmv = sm.tile([128, BC], F32, tag="tmv", name="tmv")
        nc.vector.tensor_tensor(tmv[:], mv_t[:], qp32[:, BC:2 * BC], ALU.mult)
        ps_c = ps_sm.tile([1, BC], F32, tag="ps_small", name="ps_crow")
        nc.tensor.matmul(ps_c[:], onesc_sb[:], tmv[:])
        c_row = consts.tile([1, BC], F32, tag="crow", name="c_row")
        nc.scalar.activation(c_row[:], ps_c[:], ACTF.Copy)
        ps_cb = ps_sm.tile([128, BC], F32, tag="ps_small", name="ps_cbc")
        nc.tensor.matmul(ps_cb[:], onesr_sb[:], c_row[:])
        c_bc = consts.tile([128, BC], F32, tag="cbc", name="c_bc")
        nc.scalar.activation(c_bc[:], ps_cb[:], ACTF.Copy)

        # accumulators
        ro_t = sm.tile([128, BC], F32, tag="rot", name="ro_t", bufs=1)
        ps_srow = ps_sm.tile([1, BC], F32, tag="ps_srow", name="ps_srow",
                             bufs=1)

        lt_tiles = [None] * BC
        g_state = [None] * BC

        def stage1(b):
            # pass 1: col-tiled logits + fix-up -> lt (PE + copies)
            mb = m_tiles[b]
            lt = ltp.tile([128, 1024], F32, tag="lt", name=f"lt{b}")
            lt_tiles[b] = lt
            for r in range(2):
                ps = ps_ct.tile([128, 512], F32, tag="ps_ct", name=f"psct{b}_{r}")
                for j in range(4):
                    nc.tensor.matmul(
                        ps[32 * j:32 * j + 32, :], qp32[:],
                        mb[:, (4 * r + j) * 512:(4 * r + j + 1) * 512],
                        start=True, stop=True, tile_position=(0, 32 * j))
                ct = ctp.tile([128, 512], F32, tag="ct", name=f"ct{b}_{r}")
                nc.any.tensor_copy(ct[:], ps[:])
                psf = ps_fx.tile([128, 512], F32, tag="ps_fx", name=f"psfx{b}_{r}")
                for q4 in range(4):
                    nc.tensor.matmul(psf[:, q4 * 128:(q4 + 1) * 128],
                                     ct[:, q4 * 128:(q4 + 1) * 128],
                                     ident_sb[:], is_transpose=True)
                nc.any.tensor_copy(
                    lt[:, r * 512:(r + 1) * 512].rearrange(
                        "p (j q2 q) -> p q2 j q", j=4, q2=4, q=32),
                    psf[:].rearrange("p (q2 j q) -> p q2 j q", q2=4, j=4, q=32))

        def stage2(b):
            # softmax chain -> g, fold to g_row
            lt = lt_tiles[b]
            wl = lt[:, b::32]
            lr = lt[:, (8 + b)::32]

            e1 = sm.tile([128, 32], F32, tag="e1", name=f"e1_{b}")
            e1s = sm.tile([128, 1], F32, tag="e1s", name=f"e1s_{b}")
            nc.scalar.activation(e1[:], wl, ACTF.Exp, accum_out=e1s[:])
            ps_s1 = ps_sm.tile([1, 1], F32, tag="ps_small", name=f"ps_s1_{b}")
            nc.tensor.matmul(ps_s1[:], e1s[:], onesc_sb[:, 0:1])
            s1 = sm.tile([1, 1], F32, tag="s1", name=f"s1_{b}")
            nc.any.tensor_copy(s1[:], ps_s1[:])
            r1 = sm.tile([1, 1], F32, tag="r1", name=f"r1_{b}")
            nc.vector.reciprocal(r1[:], s1[:])
            ps_r1 = ps_sm.tile([128, 1], F32, tag="ps_small", name=f"ps_r1_{b}")
            nc.tensor.matmul(ps_r1[:], onesr_sb[:], r1[:])
            r1c = sm.tile([128, 1], F32, tag="r1c", name=f"r1c_{b}")
            nc.any.tensor_copy(r1c[:], ps_r1[:])
            ww = sm.tile([128, 32], F32, tag="ww", name=f"ww_{b}")
            nc.vector.tensor_scalar_mul(ww[:], e1[:], r1c[:])

            t1 = sm.tile([128, 32], F32, tag="t1", name=f"t1_{b}")
            nc.vector.scalar_tensor_tensor(
                t1[:], lr, c_bc[:, b:b + 1], ww[:],
                op0=ALU.subtract, op1=ALU.mult)
            rl = sm.tile([128, 32], F32, tag="rl", name=f"rl_{b}")
            nc.vector.scalar_tensor_tensor(
                rl[:], lr, 0.0, t1[:], op0=ALU.add, op1=ALU.subtract)

            e2 = sm.tile([128, 32], F32, tag="e2", name=f"e2_{b}")
            e2s = sm.tile([128, 1], F32, tag="e2s", name=f"e2s_{b}")
            nc.scalar.activation(e2[:], rl[:], ACTF.Exp, accum_out=e2s[:])
            ps_s2 = ps_sm.tile([1, 1], F32, tag="ps_small", name=f"ps_s2_{b}")
            nc.tensor.matmul(ps_s2[:], e2s[:], onesc_sb[:, 0:1])
            s2 = sm.tile([1, 1], F32, tag="s2", name=f"s2_{b}")
            nc.any.tensor_copy(s2[:], ps_s2[:])
            r2 = sm.tile([1, 1], F32, tag="r2", name=f"r2_{b}")
            nc.vector.reciprocal(r2[:], s2[:])
            ps_r2 = ps_sm.tile([128, 1], F32, tag="ps_small", name=f"ps_r2_{b}")
            nc.tensor.matmul(ps_r2[:], onesr_sb[:], r2[:])
            r2c = sm.tile([128, 1], F32, tag="r2c", name=f"r2c_{b}")
            nc.any.tensor_copy(r2c[:], ps_r2[:])
            rw = sm.tile([128, 32], F32, tag="rw", name=f"rw_{b}")
            nc.vector.tensor_scalar_mul(rw[:], e2[:], r2c[:])

            t2 = sm.tile([128, 32], F32, tag="t2", name=f"t2_{b}")
            nc.vector.tensor_tensor(t2[:], rw[:], ww[:], ALU.mult)
            g = sm.tile([128, 32], F32, tag="g", name=f"g_{b}")
            nc.vector.tensor_tensor(g[:], rw[:], t2[:], ALU.subtract)
            t2s = sm.tile([128, 1], F32, tag="t2s", name=f"t2s_{b}")
            nc.vector.tensor_reduce(t2s[:], t2[:], AX.X, ALU.add)
            nc.tensor.matmul(ps_srow[0:1, b:b + 1], t2s[:], onesc_sb[:, 0:1])

            # n = 128*k' + nsub with k' = 16r + 4j + Q = g's compact col.
            ps_gt = ps_sm.tile([32, 128], F32, tag="ps_small",
                               name=f"ps_gt_{b}")
            nc.tensor.matmul(ps_gt[:], g[:], ident_sb[:], is_transpose=True)
            gt = sm.tile([32, 128], F32, tag="gt", name=f"gt_{b}")
            nc.any.tensor_copy(gt[:], ps_gt[:])
            g_row = grp.tile([1, N_SLOTS], gdt, tag="grow", name=f"grow_{b}",
                             bufs=2)
            nc.gpsimd.dma_start(g_row[0:1, :], gt[:])
            g_state[b] = g_row

        def stage3(b):
            # pass 2: per 512-quarter GB bcast (PE) + stt (DVE)
            mb = m_tiles[b]
            g_row = g_state[b]
            acc = sm.tile([128, 1], F32, tag="acc", name=f"acc_{b}")
            sttout = ctp.tile([128, 512], F32, tag="sttout", name=f"so_{b}")
            for qi in range(NGRP):
                psg = ps_gb.tile([128, 512], F32, tag="ps_gb",
                                 name=f"psgb{b}_{qi}")
                nc.tensor.matmul(psg[:], onesr_r[:],
                                 g_row[0:1, qi * 512:(qi + 1) * 512])
                acc_q = sm.tile([128, 1], F32, tag="accq",
                                name=f"accq_{b}_{qi}")
                nc.vector.scalar_tensor_tensor(
                    sttout[:], mb[:, qi * 512:(qi + 1) * 512], 1.0, psg[:],
                    op0=ALU.mult, op1=ALU.mult, accum_out=acc_q[:])
                if qi == 0:
                    nc.vector.tensor_copy(acc[:], acc_q[:])
                else:
                    nc.vector.tensor_tensor(acc[:], acc[:], acc_q[:], ALU.add)
            nc.vector.tensor_copy(ro_t[:, b:b + 1], acc[:])

        if phase == "p1":
            for b in range(BC):
                stage1(b)
            return
        if phase == "sm":
            for t in range(BC + 1):
                if t < BC:
                    stage1(t)
                if t >= 1:
                    stage2(t - 1)
            return
        for t in range(BC + 2):
            if t < BC:
                stage1(t)
            if t >= 2:
                stage3(t - 2)
            if t >= 1 and t - 1 < BC:
                stage2(t - 1)

        # ---------- epilogue ----------
        if phase != "full":
            return
        s_row = sm.tile([1, BC], F32, tag="srow", name="s_row")
        nc.any.tensor_copy(s_row[:], ps_srow[:])
        ps_sbc = ps_sm.tile([128, BC], F32, tag="ps_small", name="ps_sbc")
        nc.tensor.matmul(ps_sbc[:], onesr_sb[:], s_row[:])
        s_bc = sm.tile([128, BC], F32, tag="sbc", name="s_bc")
        nc.any.tensor_copy(s_bc[:], ps_sbc[:])

        t3 = sm.tile([128, BC], F32, tag="t3", name="t3")
        nc.vector.tensor_tensor(t3[:], mv_t[:], s_bc[:], ALU.mult)
        ro2 = sm.tile([128, BC], F32, tag="ro2", name="ro2")
        nc.vector.tensor_tensor(ro2[:], ro_t[:], t3[:], ALU.add)

        ps_out = ps_sm.tile([BC, IN_DIM], F32, tag="ps_small", name="ps_out")
        nc.tensor.matmul(ps_out[:], onesr_sb[:, 0:BC], b_ro_sb[:],
                         start=True, stop=False)
        nc.tensor.matmul(ps_out[:], ro2[:], w_ro_sb[:], start=False, stop=True)
        out_sb = sm.tile([BC, IN_DIM], F32, tag="outsb", name="out_sb")
        nc.any.tensor_copy(out_sb[:], ps_out[:])
        nc.sync.dma_start(out, out_sb[:])


_NC_CACHE = None


def _get_nc():
    global _NC_CACHE
    if _NC_CACHE is None:
        _NC_CACHE = build_nc()
    return _NC_CACHE


def make_in_maps(inputs):
    ident = np.eye(128, dtype=np.float32)
    ones_col = np.ones((128, 1), dtype=np.float32)
    ones_row = np.ones((1, 128), dtype=np.float32)
    shared = {
        "W_write": np.ascontiguousarray(inputs["W_write"], dtype=np.float32),
        "b_write": np.ascontiguousarray(inputs["b_write"], dtype=np.float32).reshape(1, D),
        "W_wq": np.ascontiguousarray(inputs["W_wq"], dtype=np.float32),
        "b_wq": np.ascontiguousarray(inputs["b_wq"], dtype=np.float32).reshape(1, D),
        "W_rq": np.ascontiguousarray(inputs["W_rq"], dtype=np.float32),
        "b_rq": np.ascontiguousarray(inputs["b_rq"], dtype=np.float32).reshape(1, D),
        "W_ro": np.ascontiguousarray(inputs["W_ro"], dtype=np.float32),
        "b_ro": np.ascontiguousarray(inputs["b_ro"], dtype=np.float32).reshape(1, IN_DIM),
        "ident": ident, "ones_col": ones_col, "ones_row": ones_row,
    }
    x = np.ascontiguousarray(inputs["x"], dtype=np.float32)
    memt = np.ascontiguousarray(
        np.asarray(inputs["memory"], dtype=np.float32).transpose(0, 2, 1))
    in_maps = []
    for i in range(N_CORES):
        m = dict(shared)
        m["x"] = np.ascontiguousarray(x[i * BC:(i + 1) * BC])
        m["memoryT"] = np.ascontiguousarray(memt[i * BC:(i + 1) * BC])
        in_maps.append(m)
    return in_maps


def kernel(**inputs) -> np.ndarray:
    nc = _get_nc()
    in_maps = make_in_maps(inputs)
    res = run_bass_kernel_spmd(nc, in_maps, list(range(N_CORES)))
    out = np.concatenate([res.results[i]["out"] for i in range(N_CORES)], axis=0)
    return np.ascontiguousarray(out, dtype=np.float32)


if __name__ == "__main__":
    nc = build_nc()
    print("built ok; instructions:",
          sum(len(bb.instructions) for bb in nc.main_func.blocks))



# revision 48
# speedup vs baseline: 1.7989x; 1.7989x over previous
"""AttentionMemoryInterface Trainium2 kernel (v6).

Reference computation per batch element b (memory [N=4096, D=128], x [256]):
    mv = x@W_write+b_write; wq = x@W_wq+b_wq; rq = x@W_rq+b_rq
    wl[n] = mem[n,:]@wq ; ww = softmax(wl)
    new_mem = mem*(1-ww) + mv*ww
    rl[n] = new_mem[n,:]@rq ; rw = softmax(rl)
    out = (rw @ new_mem) @ W_ro + b_ro

Algebraic restructure (new_mem never materialized):
    lr[n] = mem[n,:]@rq                  (same pass as wl)
    cbar  = mv@rq                        (scalar per b)
    rl[n] = lr[n] + ww[n]*(cbar - lr[n])
    g[n]  = rw[n]*(1-ww[n]);  s = sum_n rw[n]*ww[n]
    read_out = sum_n g[n]*mem[n,:] + s*mv
    out = read_out @ W_ro + b_ro

v6 structure (per core, 8 batch elements, data-parallel over batch):
  - all small constants (weights, biases, pre-transposed x, ones, ident)
    packed host-side into one [128,1936] blob -> a single DMA, killing
    the per-DMA HWDGE serialization that stalled startup by ~10us.
  - memT [128(d), 4096(n)] tiles DMA'd in 8 chunks each so small DMAs
    (g-row folds) can slot into the serialized DMA-engine queue.
  - PE p-state warm-up burst of dummy matmuls while the first memory
    tiles stream in (the cost model's tensor engine clocks up only
    after sustained work).
  - pass 1: per 128-wide n-block, one matmul with STATIONARY = memT
    block [128(d),128(n)] and MOVING = (wq_b|rq_b) [128,2] -> PSUM
    [128(n_sub), 64] directly in softmax-friendly layout (col 2k = wl
    block k, col 2k+1 = lr block k). Output free size is 2, so pass-1
    PE time is negligible and no logit fix-up transposes are needed.
  - softmax: logits staged to SBUF once per b, then v3-proven ops only:
    ACT exp with accum_out, PE ones-matmul column sums + broadcasts,
    DVE reciprocal / stt / tsm. The read-softmax normalization (1/S2)
    is deferred and applied once in the epilogue.
  - g: PE transpose -> ACT copy -> gpsimd SWDGE fold (casting f32 ->
    f32r) -> g_row [1,4096] in n order, issued with high_priority so
    the tile scheduler doesn't starve it behind the memT chunk DMAs.
  - pass 2: PE broadcasts of g_row (f32r moving, 1 cycle/row) fill
    [128,1024] two-bank PSUM pairs; DVE scalar_tensor_tensor
    (memT * bcast) with accum_out, one op per pair (halves the
    per-instruction PSUM-access overhead); one column-reduce per b.
"""

import numpy as np

import concourse.bass as bass
import concourse.bass_isa as bass_isa
import concourse.bacc as bacc
import concourse.mybir as mybir
import concourse.tile as tile
from concourse.bass_utils import run_bass_kernel_spmd

N_CORES = 8
B, IN_DIM, D, N_SLOTS = 64, 256, 128, 4096
BC = B // N_CORES          # batch per core
NBLK = N_SLOTS // 128      # 32 n-blocks per b
NGRP = N_SLOTS // 512      # 8 pass-2 quarters per b
F32 = mybir.dt.float32
F32R = mybir.dt.float32r
BF16 = mybir.dt.bfloat16
AX = mybir.AxisListType
ALU = mybir.AluOpType
ACTF = mybir.ActivationFunctionType
RED = bass_isa.ReduceOp

# const blob column offsets
C_WWR = 0            # 2 x [128,128]
C_WWQ = 256
C_WRQ = 512
C_WRO = 768          # [128,256]
C_XT = 1024          # 2 x [128,8]
C_ONES = 1040        # [128,128] all ones (row 0 used as [1,128])
C_BWR = 1168         # [1,128] each (row 0)
C_BWQ = 1296
C_BRQ = 1424
C_BRO = 1552         # [1,256]
C_IDENT = 1808       # [128,128] identity (PE transpose)
C_TOT = 1936


def build_nc(loop_n: int = 1, phase: str = "full", dve_q: int = 5,
             n_warm: int = 6, n_fill: int = 0):
    nc = bacc.Bacc("TRN2", target_bir_lowering=False, debug=False,
                   num_devices=N_CORES)

    blob_d = nc.dram_tensor("blob", [128, C_TOT], F32, kind="ExternalInput")
    memt_d = nc.dram_tensor("memoryT", [BC, D, N_SLOTS], F32,
                            kind="ExternalInput")
    out_d = nc.dram_tensor("out", [BC, IN_DIM], F32, kind="ExternalOutput")

    kw = dict(blob=blob_d.ap(), memt=memt_d.ap(), out=out_d.ap(),
              phase=phase, dve_q=dve_q, n_warm=n_warm, n_fill=n_fill)
    with tile.TileContext(nc) as tc:
        if loop_n == 1:
            _body(nc, tc, **kw)
        else:
            with tc.For_i(0, loop_n, 1):
                _body(nc, tc, **kw)
    nc.compile()
    return nc


def _body(nc, tc, *, blob, memt, out, phase, dve_q, n_warm, n_fill):
    from contextlib import ExitStack
    ctx = ExitStack()
    with ctx:
        consts = ctx.enter_context(tc.tile_pool(name="consts", bufs=1))
        mtp = ctx.enter_context(tc.tile_pool(name="mt", bufs=1))
        sm = ctx.enter_context(tc.tile_pool(name="sm", bufs=2))
        grp = ctx.enter_context(tc.tile_pool(name="gr", bufs=3))
        trs = ctx.enter_context(tc.tile_pool(name="trs", bufs=2))
        ps_lt = ctx.enter_context(tc.tile_pool(name="ps_lt", bufs=2, space="PSUM"))
        ps_gb2 = ctx.enter_context(tc.tile_pool(name="ps_gb2", bufs=2, space="PSUM"))
        ps_sm = ctx.enter_context(tc.tile_pool(name="ps_sm", bufs=1, space="PSUM"))

        # ---------- const blob (one DMA) ----------
        cb = consts.tile([128, C_TOT], F32, tag="blob", name="cb")
        nc.sync.dma_start(cb[:], blob)
        ones_r = cb[0:1, C_ONES:C_ONES + 128]
        # f32r copy of the ones row (DMA cast rounds; bitcast is rejected by
        # the BIR verifier for f32r matmul operands)
        ones_rr = consts.tile([1, 128], F32R, tag="onesrr", name="ones_rr")
        nc.gpsimd.dma_start(ones_rr[:], blob[0:1, C_ONES:C_ONES + 128])

        # ---------- memory DMAs (flat, per-partition contiguous) ----------
        # chunked so small DMAs (g_row folds, out) can slot into the
        # serialized DMA-engine queue between transfers
        m_tiles = []
        MCH = 8
        assert N_SLOTS % MCH == 0 or True
        for b in range(BC):
            mb = mtp.tile([128, N_SLOTS], F32, tag=f"mem{b}", name=f"memt{b}")
            bounds = [N_SLOTS * c // MCH for c in range(MCH + 1)]
            for c in range(MCH):
                nc.sync.dma_start(mb[:, bounds[c]:bounds[c + 1]],
                                  memt[b][:, bounds[c]:bounds[c + 1]])
            m_tiles.append(mb)

        if phase == "dma":
            dummy = sm.tile([128, BC], F32, tag="dummy", name="dummy")
            for b in range(BC):
                nc.vector.tensor_copy(dummy[:, b:b + 1], m_tiles[b][:, 0:1])
            return

        # ---------- PE p-state warm-up (dummy f32 matmuls) ----------
        for i in range(n_warm):
            psw = ps_gb2.tile([128, 1024], F32, tag="ps_gb2", name=f"warm{i}")
            nc.tensor.matmul(psw[:, 0:128], ones_r,
                             cb[0:1, C_WWQ:C_WWQ + 128])

        # ---------- projections -> mv_t [128, BC], qp [128, 2*BC] ----------
        # qp columns: col 2b = wq_b, col 2b+1 = rq_b.
        mv_t = consts.tile([128, BC], F32, tag="mvt", name="mv_t")
        qp = consts.tile([128, 2 * BC], F32, tag="qp", name="qp")
        for j, (cw, cbias) in enumerate(((C_WWR, C_BWR), (C_WWQ, C_BWQ),
                                         (C_WRQ, C_BRQ))):
            ps = ps_sm.tile([128, BC], F32, tag="ps_small", name=f"ps_proj{j}")
            nc.tensor.matmul(ps[:], cb[0:1, cbias:cbias + D],
                             ones_r[0:1, 0:BC], start=True, stop=False)
            for k in range(IN_DIM // 128):
                nc.tensor.matmul(
                    ps[:], cb[:, cw + k * 128:cw + (k + 1) * 128],
                    cb[:, C_XT + k * BC:C_XT + (k + 1) * BC],
                    start=False, stop=(k == IN_DIM // 128 - 1))
            if j == 0:
                nc.scalar.activation(mv_t[:], ps[:], ACTF.Copy)
            else:
                nc.scalar.activation(qp[:, (j - 1)::2], ps[:], ACTF.Copy)

        # ---------- cbar (mv . rq per b) ----------
        ones_c = cb[:, C_ONES:C_ONES + 1]
        tmv = sm.tile([128, BC], F32, tag="tmv", name="tmv")
        nc.vector.tensor_tensor(tmv[:], mv_t[:], qp[:, 1::2], ALU.mult)
        ps_c = ps_sm.tile([1, BC], F32, tag="ps_small", name="ps_crow")
        nc.tensor.matmul(ps_c[:], ones_c, tmv[:])
        c_row = consts.tile([1, BC], F32, tag="crow", name="c_row")
        nc.scalar.activation(c_row[:], ps_c[:], ACTF.Copy)
        ps_cb = ps_sm.tile([128, BC], F32, tag="ps_small", name="ps_cbc")
        nc.tensor.matmul(ps_cb[:], ones_r, c_row[:])
        c_bc = consts.tile([128, BC], F32, tag="cbc", name="c_bc")
        nc.scalar.activation(c_bc[:], ps_cb[:], ACTF.Copy)

        # accumulators / per-b scalar rows (bcast once in the epilogue)
        ro_t = sm.tile([128, BC], F32, tag="rot", name="ro_t", bufs=1)
        ps_rows = ps_sm.tile([1, 2 * BC], F32, tag="ps_rows", name="ps_rows",
                             bufs=1)
        ps_srow = ps_rows[0:1, 0:BC]
        ps_s2row = ps_rows[0:1, BC:2 * BC]

        lt_tiles = [None] * BC
        g_state = [None] * BC

        def stage1(b):
            # logits: stationary = memT n-block (f32r), moving = (wq_b|rq_b)
            mb = m_tiles[b]
            ps = ps_lt.tile([128, 2 * NBLK], F32, tag="ps_lt", name=f"pslt{b}")
            lt_tiles[b] = ps
            q2 = qp[:, 2 * b:2 * b + 2]
            for k in range(NBLK):
                nc.tensor.matmul(ps[:, 2 * k:2 * k + 2],
                                 mb[:, 128 * k:128 * (k + 1)],
                                 q2, start=True, stop=True)

        def stage2(b):
            # softmax chain -> unnormalized g (missing 1/S2), s' col
            ps = lt_tiles[b]
            wl = ps[:, 0::2]
            lr = ps[:, 1::2]

            e1 = sm.tile([128, NBLK], F32, tag="e1", name=f"e1_{b}")
            e1s = sm.tile([128, 1], F32, tag="e1s", name=f"e1s_{b}")
            nc.scalar.activation(e1[:], wl, ACTF.Exp, accum_out=e1s[:])
            s1a = sm.tile([128, 1], F32, tag="s1a", name=f"s1a_{b}")
            nc.gpsimd.partition_all_reduce(s1a[:], e1s[:], channels=128,
                                           reduce_op=RED.add)
            r1c = sm.tile([128, 1], F32, tag="r1c", name=f"r1c_{b}")
            nc.vector.reciprocal(r1c[:], s1a[:])

            # ww = e1/S1 ; t1 = (lr - cbar)*ww ; rl = lr - t1
            ww = sm.tile([128, NBLK], F32, tag="ww", name=f"ww_{b}")
            nc.vector.tensor_scalar_mul(ww[:], e1[:], r1c[:, 0:1])
            t1 = sm.tile([128, NBLK], F32, tag="t1", name=f"t1_{b}")
            nc.vector.scalar_tensor_tensor(
                t1[:], lr, c_bc[:, b:b + 1], ww[:],
                op0=ALU.subtract, op1=ALU.mult)
            rl = sm.tile([128, NBLK], F32, tag="rl", name=f"rl_{b}")
            nc.vector.scalar_tensor_tensor(
                rl[:], lr, 0.0, t1[:], op0=ALU.add, op1=ALU.subtract)

            e2 = sm.tile([128, NBLK], F32, tag="e2", name=f"e2_{b}")
            e2s = sm.tile([128, 1], F32, tag="e2s", name=f"e2s_{b}")
            nc.scalar.activation(e2[:], rl[:], ACTF.Exp,
                                 accum_out=e2s[:])
            s2a = sm.tile([128, 1], F32, tag="s2a", name=f"s2a_{b}")
            nc.gpsimd.partition_all_reduce(s2a[:], e2s[:], channels=128,
                                           reduce_op=RED.add)
            nc.vector.reciprocal(rcol[:, b:b + 1], s2a[:])

            # t2 = e2*ww ; g = e2 - t2. On Pool/ACT so the g -> fold path
            # never queues behind the previous batch's big DVE stt block.
            t2 = sm.tile([128, NBLK], F32, tag="t2", name=f"t2_{b}")
            nc.gpsimd.tensor_tensor(t2[:], e2[:], ww[:], ALU.mult)
            t2c = sm.tile([128, NBLK], F32, tag="t2c", name=f"t2c_{b}")
            t2s = sm.tile([128, 1], F32, tag="t2s", name=f"t2s_{b}")
            nc.scalar.activation(t2c[:], t2[:], ACTF.Copy, accum_out=t2s[:])
            g = sm.tile([128, NBLK], F32, tag="g", name=f"g_{b}")
            nc.gpsimd.tensor_tensor(g[:], e2[:], t2[:], ALU.subtract)
            s3a = sm.tile([128, 1], F32, tag="s3a", name=f"s3a_{b}")
            nc.gpsimd.partition_all_reduce(s3a[:], t2s[:], channels=128,
                                           reduce_op=RED.add)
            nc.vector.tensor_copy(scol[:, b:b + 1], s3a[:])

            # g [128(nsub), 32(k)] -> PE transpose -> gt [32(k), 128(nsub)]
            # (bf16, so the fold needs no cast and the g-broadcast matmuls
            # run at 1 cyc/row) -> two natural-AP half folds -> g_row
            # [1,4096] in n order (n = 128k + nsub).
            ps_gt = ps_sm.tile([NBLK, 128], F32, tag="ps_small",
                               name=f"ps_gt_{b}")
            nc.tensor.matmul(ps_gt[:], g[:],
                             cb[:, C_IDENT:C_IDENT + 128],
                             is_transpose=True)
            gt = sm.tile([NBLK, 128], F32, tag="gt", name=f"gt_{b}")
            nc.scalar.activation(gt[:], ps_gt[:], ACTF.Copy)
            g_row = grp.tile([1, N_SLOTS], F32R, tag="grow", name=f"grow_{b}",
                             bufs=3)
            # SWDGE (gpsimd) fold: casts f32 -> f32r while rearranging
            # partitions into one row (the proven v3 pattern)
            with tc.high_priority():
                nc.gpsimd.dma_start(g_row[0:1, :], gt[:])
            g_state[b] = g_row

        def stage3(b):
            # pass 2: weighted sum over n, all on DVE as 4 stt ops over
            # [128,1024] PSUM pairs (GPSIMD can't read PSUM / run stt, and
            # mixing Pool tensor ops with partition_all_reduce would thrash
            # Q7 ucode library reloads).
            mb = m_tiles[b]
            g_row = g_state[b]
            acc8 = sm.tile([128, 4], F32, tag="acc8", name=f"acc8_{b}")
            for p in range(4):
                psg2 = ps_gb2.tile([128, 1024], F32, tag="ps_gb2",
                                   name=f"psgb2_{b}_{p}")
                for h in range(2):
                    qi = 2 * p + h
                    nc.tensor.matmul(
                        psg2[:, h * 512:(h + 1) * 512], ones_rr[:],
                        g_row[0:1, qi * 512:(qi + 1) * 512])
                trash = trs.tile([128, 1024], F32, tag="trash0",
                                 name=f"trash_{b}_{p}")
                nc.vector.scalar_tensor_tensor(
                    trash[:], mb[:, p * 1024:(p + 1) * 1024], 1.0, psg2[:],
                    op0=ALU.mult, op1=ALU.mult, accum_out=acc8[:, p:p + 1])
            nc.vector.tensor_reduce(ro_t[:, b:b + 1], acc8[:], AX.X, ALU.add)

        if phase == "p1":
            for b in range(BC):
                stage1(b)
            return
        if phase == "sm":
            for t in range(BC + 1):
                if t < BC:
                    stage1(t)
                if t >= 1:
                    stage2(t - 1)
            return
        # issue order per slot: stage1(t) [PE], stage2(t) [ACT/DVE/Pool
        # chain], stage3(t-1) [PE bcasts + DVE/Pool stt]. stage2(t) before
        # stage3(t-1) keeps each engine's in-order stream aligned with data
        # readiness (chain ops for t become ready before t-1's stt).
        for t in range(BC + 1):
            if t < BC:
                stage1(t)
                stage2(t)
            if t >= 1:
                stage3(t - 1)

        # ---------- PE p-state keep-warm fillers ----------
        # No-dep worst-priority matmuls: the tile scheduler drips them into
        # every PE idle gap, keeping the p-state ramp from resetting between
        # real bursts. Issued last so all real work outranks them.
        if n_fill:
            psf = ps_sm.tile([128, 64], F32, tag="ps_fill", name="ps_fill",
                             bufs=1)
            for i in range(n_fill):
                nc.tensor.matmul(psf[:], ones_r,
                                 cb[0:1, C_WWR:C_WWR + 64])

        # ---------- epilogue ----------
        if phase != "full":
            return
        # ro3 = (ro_t + mv*s') / S2   (deferred read-softmax normalization)
        s_row = sm.tile([1, BC], F32, tag="s_row", name="s_row")
        nc.scalar.activation(s_row[:], ps_srow, ACTF.Copy)
        ps_sb = ps_sm.tile([128, BC], F32, tag="ps_small", name="ps_sb")
        nc.tensor.matmul(ps_sb[:], ones_r, s_row[:])
        s_bc = sm.tile([128, BC], F32, tag="s_bc", name="s_bc")
        nc.scalar.activation(s_bc[:], ps_sb[:], ACTF.Copy)
        s2_row = sm.tile([1, BC], F32, tag="s2_row", name="s2_row")
        nc.scalar.activation(s2_row[:], ps_s2row, ACTF.Copy)
        r2_row = sm.tile([1, BC], F32, tag="r2_row", name="r2_row")
        nc.vector.reciprocal(r2_row[:], s2_row[:])
        ps_rb = ps_sm.tile([128, BC], F32, tag="ps_small", name="ps_rb")
        nc.tensor.matmul(ps_rb[:], ones_r, r2_row[:])
        r_bc = sm.tile([128, BC], F32, tag="r_bc", name="r_bc")
        nc.scalar.activation(r_bc[:], ps_rb[:], ACTF.Copy)
        t3 = sm.tile([128, BC], F32, tag="t3", name="t3")
        nc.vector.tensor_tensor(t3[:], mv_t[:], s_bc[:], ALU.mult)
        ro2 = sm.tile([128, BC], F32, tag="ro2", name="ro2")
        nc.vector.tensor_tensor(ro2[:], ro_t[:], t3[:], ALU.add)
        ro3 = sm.tile([128, BC], F32, tag="ro3", name="ro3")
        nc.vector.tensor_tensor(ro3[:], ro2[:], r_bc[:], ALU.mult)

        ps_out = ps_sm.tile([BC, IN_DIM], F32, tag="ps_small", name="ps_out")
        nc.tensor.matmul(ps_out[:], ones_r[0:1, 0:BC],
                         cb[0:1, C_BRO:C_BRO + IN_DIM],
                         start=True, stop=False)
        nc.tensor.matmul(ps_out[:], ro3[:],
                         cb[:, C_WRO:C_WRO + IN_DIM],
                         start=False, stop=True)
        out_sb = sm.tile([BC, IN_DIM], F32, tag="outsb", name="out_sb")
        nc.vector.tensor_copy(out_sb[:], ps_out[:])
        nc.sync.dma_start(out, out_sb[:])


_NC_CACHE = None


def _get_nc():
    global _NC_CACHE
    if _NC_CACHE is None:
        _NC_CACHE = build_nc()
    return _NC_CACHE


def make_blob(inputs, core):
    blob = np.zeros((128, C_TOT), dtype=np.float32)
    W = {k: np.ascontiguousarray(inputs[k], dtype=np.float32)
         for k in ("W_write", "W_wq", "W_rq", "W_ro")}
    for cw, k in ((C_WWR, "W_write"), (C_WWQ, "W_wq"), (C_WRQ, "W_rq")):
        blob[:, cw:cw + 128] = W[k][0:128, :]
        blob[:, cw + 128:cw + 256] = W[k][128:256, :]
    blob[:, C_WRO:C_WRO + IN_DIM] = W["W_ro"]
    xt = np.ascontiguousarray(
        np.asarray(inputs["x"], dtype=np.float32)[core * BC:(core + 1) * BC]).T
    blob[:, C_XT:C_XT + BC] = xt[0:128, :]
    blob[:, C_XT + BC:C_XT + 2 * BC] = xt[128:256, :]
    blob[:, C_ONES:C_ONES + 128] = 1.0
    blob[:, C_IDENT:C_IDENT + 128] = np.eye(128, dtype=np.float32)
    blob[0, C_BWR:C_BWR + D] = np.asarray(inputs["b_write"], dtype=np.float32)
    blob[0, C_BWQ:C_BWQ + D] = np.asarray(inputs["b_wq"], dtype=np.float32)
    blob[0, C_BRQ:C_BRQ + D] = np.asarray(inputs["b_rq"], dtype=np.float32)
    blob[0, C_BRO:C_BRO + IN_DIM] = np.asarray(inputs["b_ro"], dtype=np.float32)
    return blob


def make_in_maps(inputs):
    memt = np.ascontiguousarray(
        np.asarray(inputs["memory"], dtype=np.float32).transpose(0, 2, 1))
    in_maps = []
    for i in range(N_CORES):
        m = {"blob": make_blob(inputs, i),
             "memoryT": np.ascontiguousarray(memt[i * BC:(i + 1) * BC])}
        in_maps.append(m)
    return in_maps


def kernel(**inputs) -> np.ndarray:
    nc = _get_nc()
    in_maps = make_in_maps(inputs)
    res = run_bass_kernel_spmd(nc, in_maps, list(range(N_CORES)))
    out = np.concatenate([res.results[i]["out"] for i in range(N_CORES)], axis=0)
    return np.ascontiguousarray(out, dtype=np.float32)


if __name__ == "__main__":
    nc = build_nc()
    print("built ok; instructions:",
          sum(len(bb.instructions) for bb in nc.main_func.blocks))


# revision 55
# speedup vs baseline: 1.8213x; 1.0125x over previous
"""AttentionMemoryInterface Trainium2 kernel (v6).

Reference computation per batch element b (memory [N=4096, D=128], x [256]):
    mv = x@W_write+b_write; wq = x@W_wq+b_wq; rq = x@W_rq+b_rq
    wl[n] = mem[n,:]@wq ; ww = softmax(wl)
    new_mem = mem*(1-ww) + mv*ww
    rl[n] = new_mem[n,:]@rq ; rw = softmax(rl)
    out = (rw @ new_mem) @ W_ro + b_ro

Algebraic restructure (new_mem never materialized):
    lr[n] = mem[n,:]@rq                  (same pass as wl)
    cbar  = mv@rq                        (scalar per b)
    rl[n] = lr[n] + ww[n]*(cbar - lr[n])
    g[n]  = rw[n]*(1-ww[n]);  s = sum_n rw[n]*ww[n]
    read_out = sum_n g[n]*mem[n,:] + s*mv
    out = read_out @ W_ro + b_ro

v6 structure (per core, 8 batch elements, data-parallel over batch):
  - all small constants (weights, biases, pre-transposed x, ones, ident)
    packed host-side into one [128,1936] blob -> a single DMA, killing
    the per-DMA HWDGE serialization that stalled startup by ~10us.
  - memT [128(d), 4096(n)] tiles DMA'd in 8 chunks each so small DMAs
    (g-row folds) can slot into the serialized DMA-engine queue.
  - PE p-state warm-up burst of dummy matmuls while the first memory
    tiles stream in (the cost model's tensor engine clocks up only
    after sustained work).
  - pass 1: per 128-wide n-block, one matmul with STATIONARY = memT
    block [128(d),128(n)] and MOVING = (wq_b|rq_b) [128,2] -> PSUM
    [128(n_sub), 64] directly in softmax-friendly layout (col 2k = wl
    block k, col 2k+1 = lr block k). Output free size is 2, so pass-1
    PE time is negligible and no logit fix-up transposes are needed.
  - softmax: logits staged to SBUF once per b, then v3-proven ops only:
    ACT exp with accum_out, PE ones-matmul column sums + broadcasts,
    DVE reciprocal / stt / tsm. The read-softmax normalization (1/S2)
    is deferred and applied once in the epilogue.
  - g: PE transpose -> ACT copy -> gpsimd SWDGE fold (casting f32 ->
    f32r) -> g_row [1,4096] in n order, issued with high_priority so
    the tile scheduler doesn't starve it behind the memT chunk DMAs.
  - pass 2: PE broadcasts of g_row (f32r moving, 1 cycle/row) fill
    [128,1024] two-bank PSUM pairs; DVE scalar_tensor_tensor
    (memT * bcast) with accum_out, one op per pair (halves the
    per-instruction PSUM-access overhead); one column-reduce per b.
"""

import numpy as np

import concourse.bass as bass
import concourse.bass_isa as bass_isa
import concourse.bacc as bacc
import concourse.mybir as mybir
import concourse.tile as tile
from concourse.bass_utils import run_bass_kernel_spmd

N_CORES = 8
B, IN_DIM, D, N_SLOTS = 64, 256, 128, 4096
BC = B // N_CORES          # batch per core
NBLK = N_SLOTS // 128      # 32 n-blocks per b
NGRP = N_SLOTS // 512      # 8 pass-2 quarters per b
F32 = mybir.dt.float32
F32R = mybir.dt.float32r
BF16 = mybir.dt.bfloat16
AX = mybir.AxisListType
ALU = mybir.AluOpType
ACTF = mybir.ActivationFunctionType
RED = bass_isa.ReduceOp

# const blob column offsets
C_WWR = 0            # 2 x [128,128]
C_WWQ = 256
C_WRQ = 512
C_WRO = 768          # [128,256]
C_XT = 1024          # 2 x [128,8]
C_ONES = 1040        # [128,128] all ones (row 0 used as [1,128])
C_IDENT = 1168       # [128,128] identity (PE transpose)
C_TOT = 1296
# row-blob (biases live on partition 0 only; shipping them in the [128,*]
# blob wasted 0.33MB of serialized DMA ahead of the memory tiles)
B2_BWR = 0           # [1,128] each
B2_BWQ = 128
B2_BRQ = 256
B2_BRO = 384         # [1,256]
B2_TOT = 640


def build_nc(loop_n: int = 1, phase: str = "full", dve_q: int = 5,
             n_warm: int = 6, n_fill: int = 0):
    nc = bacc.Bacc("TRN2", target_bir_lowering=False, debug=False,
                   num_devices=N_CORES)

    blob_d = nc.dram_tensor("blob", [128, C_TOT], F32, kind="ExternalInput")
    blob2_d = nc.dram_tensor("blob2", [1, B2_TOT], F32, kind="ExternalInput")
    memt_d = nc.dram_tensor("memoryT", [BC, D, N_SLOTS], F32,
                            kind="ExternalInput")
    out_d = nc.dram_tensor("out", [BC, IN_DIM], F32, kind="ExternalOutput")

    kw = dict(blob=blob_d.ap(), blob2=blob2_d.ap(), memt=memt_d.ap(),
              out=out_d.ap(),
              phase=phase, dve_q=dve_q, n_warm=n_warm, n_fill=n_fill)
    with tile.TileContext(nc) as tc:
        if loop_n == 1:
            _body(nc, tc, **kw)
        else:
            with tc.For_i(0, loop_n, 1):
                _body(nc, tc, **kw)
    nc.compile()
    return nc


def _body(nc, tc, *, blob, blob2, memt, out, phase, dve_q, n_warm, n_fill):
    from contextlib import ExitStack
    ctx = ExitStack()
    with ctx:
        consts = ctx.enter_context(tc.tile_pool(name="consts", bufs=1))
        mtp = ctx.enter_context(tc.tile_pool(name="mt", bufs=1))
        sm = ctx.enter_context(tc.tile_pool(name="sm", bufs=2))
        grp = ctx.enter_context(tc.tile_pool(name="gr", bufs=3))
        trs = ctx.enter_context(tc.tile_pool(name="trs", bufs=2))
        ps_lt = ctx.enter_context(tc.tile_pool(name="ps_lt", bufs=2, space="PSUM"))
        ps_gb2 = ctx.enter_context(tc.tile_pool(name="ps_gb2", bufs=2, space="PSUM"))
        ps_sm = ctx.enter_context(tc.tile_pool(name="ps_sm", bufs=1, space="PSUM"))

        # ---------- const blob (one DMA) ----------
        cb = consts.tile([128, C_TOT], F32, tag="blob", name="cb")
        nc.sync.dma_start(cb[:], blob)
        ones_r = cb[0:1, C_ONES:C_ONES + 128]
        # f32r copy of the ones row (DMA cast rounds; bitcast is rejected by
        # the BIR verifier for f32r matmul operands)
        ones_rr = consts.tile([1, 128], F32R, tag="onesrr", name="ones_rr")
        nc.gpsimd.dma_start(ones_rr[:], blob[0:1, C_ONES:C_ONES + 128])
        # bias row blob on the ACT ring (off the SP/memory DMA window)
        cb2 = consts.tile([1, B2_TOT], F32, tag="blob2", name="cb2")
        nc.scalar.dma_start(cb2[:], blob2)

        # ---------- memory DMAs (flat, per-partition contiguous) ----------
        # chunked so small DMAs (g_row folds, out) can slot into the
        # serialized DMA-engine queue between transfers
        m_tiles = []
        MCH = 8
        assert N_SLOTS % MCH == 0 or True
        for b in range(BC):
            mb = mtp.tile([128, N_SLOTS], F32, tag=f"mem{b}", name=f"memt{b}")
            bounds = [N_SLOTS * c // MCH for c in range(MCH + 1)]
            for c in range(MCH):
                nc.sync.dma_start(mb[:, bounds[c]:bounds[c + 1]],
                                  memt[b][:, bounds[c]:bounds[c + 1]])
            m_tiles.append(mb)

        if phase == "dma":
            dummy = sm.tile([128, BC], F32, tag="dummy", name="dummy")
            for b in range(BC):
                nc.vector.tensor_copy(dummy[:, b:b + 1], m_tiles[b][:, 0:1])
            return

        # ---------- PE p-state warm-up (dummy f32 matmuls) ----------
        for i in range(n_warm):
            psw = ps_gb2.tile([128, 1024], F32, tag="ps_gb2", name=f"warm{i}")
            nc.tensor.matmul(psw[:, 0:128], ones_r,
                             cb[0:1, C_WWQ:C_WWQ + 128])

        # ---------- projections -> mv_t [128, BC], qp [128, 2*BC] ----------
        # qp columns: col 2b = wq_b, col 2b+1 = rq_b.
        mv_t = consts.tile([128, BC], F32, tag="mvt", name="mv_t")
        qp = consts.tile([128, 2 * BC], F32, tag="qp", name="qp")
        for j, (cw, cbias) in enumerate(((C_WWR, B2_BWR), (C_WWQ, B2_BWQ),
                                         (C_WRQ, B2_BRQ))):
            ps = ps_sm.tile([128, BC], F32, tag="ps_small", name=f"ps_proj{j}")
            nc.tensor.matmul(ps[:], cb2[0:1, cbias:cbias + D],
                             ones_r[0:1, 0:BC], start=True, stop=False)
            for k in range(IN_DIM // 128):
                nc.tensor.matmul(
                    ps[:], cb[:, cw + k * 128:cw + (k + 1) * 128],
                    cb[:, C_XT + k * BC:C_XT + (k + 1) * BC],
                    start=False, stop=(k == IN_DIM // 128 - 1))
            if j == 0:
                nc.scalar.activation(mv_t[:], ps[:], ACTF.Copy)
            else:
                nc.scalar.activation(qp[:, (j - 1)::2], ps[:], ACTF.Copy)

        # ---------- cbar (mv . rq per b) ----------
        ones_c = cb[:, C_ONES:C_ONES + 1]
        tmv = sm.tile([128, BC], F32, tag="tmv", name="tmv")
        nc.vector.tensor_tensor(tmv[:], mv_t[:], qp[:, 1::2], ALU.mult)
        ps_c = ps_sm.tile([1, BC], F32, tag="ps_small", name="ps_crow")
        nc.tensor.matmul(ps_c[:], ones_c, tmv[:])
        c_row = consts.tile([1, BC], F32, tag="crow", name="c_row")
        nc.scalar.activation(c_row[:], ps_c[:], ACTF.Copy)
        ps_cb = ps_sm.tile([128, BC], F32, tag="ps_small", name="ps_cbc")
        nc.tensor.matmul(ps_cb[:], ones_r, c_row[:])
        c_bc = consts.tile([128, BC], F32, tag="cbc", name="c_bc")
        nc.scalar.activation(c_bc[:], ps_cb[:], ACTF.Copy)

        # accumulators / per-b scalar rows (bcast once in the epilogue)
        ro_t = sm.tile([128, BC], F32, tag="rot", name="ro_t", bufs=1)
        ps_rows = ps_sm.tile([1, 2 * BC], F32, tag="ps_rows", name="ps_rows",
                             bufs=1)
        ps_srow = ps_rows[0:1, 0:BC]
        ps_s2row = ps_rows[0:1, BC:2 * BC]

        lt_tiles = [None] * BC
        g_state = [None] * BC

        def stage1(b):
            # logits: stationary = memT n-block (f32r), moving = (wq_b|rq_b)
            mb = m_tiles[b]
            ps = ps_lt.tile([128, 2 * NBLK], F32, tag="ps_lt", name=f"pslt{b}")
            lt_tiles[b] = ps
            q2 = qp[:, 2 * b:2 * b + 2]
            for k in range(NBLK):
                nc.tensor.matmul(ps[:, 2 * k:2 * k + 2],
                                 mb[:, 128 * k:128 * (k + 1)],
                                 q2, start=True, stop=True)

        def stage2(b):
            # softmax chain -> unnormalized g (missing 1/S2), s' col
            ps = lt_tiles[b]
            wl = ps[:, 0::2]
            lr = ps[:, 1::2]

            e1 = sm.tile([128, NBLK], F32, tag="e1", name=f"e1_{b}")
            e1s = sm.tile([128, 1], F32, tag="e1s", name=f"e1s_{b}")
            nc.scalar.activation(e1[:], wl, ACTF.Exp, accum_out=e1s[:])
            s1a = sm.tile([128, 1], F32, tag="s1a", name=f"s1a_{b}")
            nc.gpsimd.partition_all_reduce(s1a[:], e1s[:], channels=128,
                                           reduce_op=RED.add)
            r1c = sm.tile([128, 1], F32, tag="r1c", name=f"r1c_{b}")
            nc.vector.reciprocal(r1c[:], s1a[:])

            # ww = e1/S1 ; t1 = (lr - cbar)*ww ; rl = lr - t1
            ww = sm.tile([128, NBLK], F32, tag="ww", name=f"ww_{b}")
            nc.vector.tensor_scalar_mul(ww[:], e1[:], r1c[:, 0:1])
            t1 = sm.tile([128, NBLK], F32, tag="t1", name=f"t1_{b}")
            nc.vector.scalar_tensor_tensor(
                t1[:], lr, c_bc[:, b:b + 1], ww[:],
                op0=ALU.subtract, op1=ALU.mult)
            rl = sm.tile([128, NBLK], F32, tag="rl", name=f"rl_{b}")
            nc.vector.scalar_tensor_tensor(
                rl[:], lr, 0.0, t1[:], op0=ALU.add, op1=ALU.subtract)

            e2 = sm.tile([128, NBLK], F32, tag="e2", name=f"e2_{b}")
            e2s = sm.tile([128, 1], F32, tag="e2s", name=f"e2s_{b}")
            nc.scalar.activation(e2[:], rl[:], ACTF.Exp,
                                 accum_out=e2s[:])
            s2a = sm.tile([128, 1], F32, tag="s2a", name=f"s2a_{b}")
            nc.gpsimd.partition_all_reduce(s2a[:], e2s[:], channels=128,
                                           reduce_op=RED.add)
            nc.vector.reciprocal(rcol[:, b:b + 1], s2a[:])

            # t2 = e2*ww ; g = e2 - t2. On Pool/ACT so the g -> fold path
            # never queues behind the previous batch's big DVE stt block.
            t2 = sm.tile([128, NBLK], F32, tag="t2", name=f"t2_{b}")
            nc.gpsimd.tensor_tensor(t2[:], e2[:], ww[:], ALU.mult)
            t2c = sm.tile([128, NBLK], F32, tag="t2c", name=f"t2c_{b}")
            t2s = sm.tile([128, 1], F32, tag="t2s", name=f"t2s_{b}")
            nc.scalar.activation(t2c[:], t2[:], ACTF.Copy, accum_out=t2s[:])
            g = sm.tile([128, NBLK], F32, tag="g", name=f"g_{b}")
            nc.gpsimd.tensor_tensor(g[:], e2[:], t2[:], ALU.subtract)
            s3a = sm.tile([128, 1], F32, tag="s3a", name=f"s3a_{b}")
            nc.gpsimd.partition_all_reduce(s3a[:], t2s[:], channels=128,
                                           reduce_op=RED.add)
            nc.vector.tensor_copy(scol[:, b:b + 1], s3a[:])

            # g [128(nsub), 32(k)] -> PE transpose -> gt [32(k), 128(nsub)]
            # (bf16, so the fold needs no cast and the g-broadcast matmuls
            # run at 1 cyc/row) -> two natural-AP half folds -> g_row
            # [1,4096] in n order (n = 128k + nsub).
            ps_gt = ps_sm.tile([NBLK, 128], F32, tag="ps_small",
                               name=f"ps_gt_{b}")
            nc.tensor.matmul(ps_gt[:], g[:],
                             cb[:, C_IDENT:C_IDENT + 128],
                             is_transpose=True)
            gt = sm.tile([NBLK, 128], F32, tag="gt", name=f"gt_{b}")
            nc.scalar.activation(gt[:], ps_gt[:], ACTF.Copy)
            g_row = grp.tile([1, N_SLOTS], F32R, tag="grow", name=f"grow_{b}",
                             bufs=3)
            # SWDGE (gpsimd) fold: casts f32 -> f32r while rearranging
            # partitions into one row (the proven v3 pattern)
            with tc.high_priority():
                nc.gpsimd.dma_start(g_row[0:1, :], gt[:])
            g_state[b] = g_row

        def stage3(b):
            # pass 2: weighted sum over n, all on DVE as 4 stt ops over
            # [128,1024] PSUM pairs (GPSIMD can't read PSUM / run stt, and
            # mixing Pool tensor ops with partition_all_reduce would thrash
            # Q7 ucode library reloads).
            mb = m_tiles[b]
            g_row = g_state[b]
            acc8 = sm.tile([128, 4], F32, tag="acc8", name=f"acc8_{b}")
            for p in range(4):
                psg2 = ps_gb2.tile([128, 1024], F32, tag="ps_gb2",
                                   name=f"psgb2_{b}_{p}")
                for h in range(2):
                    qi = 2 * p + h
                    nc.tensor.matmul(
                        psg2[:, h * 512:(h + 1) * 512], ones_rr[:],
                        g_row[0:1, qi * 512:(qi + 1) * 512])
                trash = trs.tile([128, 1024], F32, tag="trash0",
                                 name=f"trash_{b}_{p}")
                nc.vector.scalar_tensor_tensor(
                    trash[:], mb[:, p * 1024:(p + 1) * 1024], 1.0, psg2[:],
                    op0=ALU.mult, op1=ALU.mult, accum_out=acc8[:, p:p + 1])
            nc.vector.tensor_reduce(ro_t[:, b:b + 1], acc8[:], AX.X, ALU.add)

        if phase == "p1":
            for b in range(BC):
                stage1(b)
            return
        if phase == "sm":
            for t in range(BC + 1):
                if t < BC:
                    stage1(t)
                if t >= 1:
                    stage2(t - 1)
            return
        # issue order per slot: stage1(t) [PE], stage2(t) [ACT/DVE/Pool
        # chain], stage3(t-1) [PE bcasts + DVE/Pool stt]. stage2(t) before
        # stage3(t-1) keeps each engine's in-order stream aligned with data
        # readiness (chain ops for t become ready before t-1's stt).
        for t in range(BC + 1):
            if t < BC:
                stage1(t)
                stage2(t)
            if t >= 1:
                stage3(t - 1)

        # ---------- PE p-state keep-warm fillers ----------
        # No-dep worst-priority matmuls: the tile scheduler drips them into
        # every PE idle gap, keeping the p-state ramp from resetting between
        # real bursts. Issued last so all real work outranks them.
        if n_fill:
            psf = ps_sm.tile([128, 64], F32, tag="ps_fill", name="ps_fill",
                             bufs=1)
            for i in range(n_fill):
                nc.tensor.matmul(psf[:], ones_r,
                                 cb[0:1, C_WWR:C_WWR + 64])

        # ---------- epilogue ----------
        if phase != "full":
            return
        # ro3 = (ro_t + mv*s') / S2   (deferred read-softmax normalization)
        s_row = sm.tile([1, BC], F32, tag="s_row", name="s_row")
        nc.scalar.activation(s_row[:], ps_srow, ACTF.Copy)
        ps_sb = ps_sm.tile([128, BC], F32, tag="ps_small", name="ps_sb")
        nc.tensor.matmul(ps_sb[:], ones_r, s_row[:])
        s_bc = sm.tile([128, BC], F32, tag="s_bc", name="s_bc")
        nc.scalar.activation(s_bc[:], ps_sb[:], ACTF.Copy)
        s2_row = sm.tile([1, BC], F32, tag="s2_row", name="s2_row")
        nc.scalar.activation(s2_row[:], ps_s2row, ACTF.Copy)
        r2_row = sm.tile([1, BC], F32, tag="r2_row", name="r2_row")
        nc.vector.reciprocal(r2_row[:], s2_row[:])
        ps_rb = ps_sm.tile([128, BC], F32, tag="ps_small", name="ps_rb")
        nc.tensor.matmul(ps_rb[:], ones_r, r2_row[:])
        r_bc = sm.tile([128, BC], F32, tag="r_bc", name="r_bc")
        nc.scalar.activation(r_bc[:], ps_rb[:], ACTF.Copy)
        t3 = sm.tile([128, BC], F32, tag="t3", name="t3")
        nc.vector.tensor_tensor(t3[:], mv_t[:], s_bc[:], ALU.mult)
        ro2 = sm.tile([128, BC], F32, tag="ro2", name="ro2")
        nc.vector.tensor_tensor(ro2[:], ro_t[:], t3[:], ALU.add)
        ro3 = sm.tile([128, BC], F32, tag="ro3", name="ro3")
        nc.vector.tensor_tensor(ro3[:], ro2[:], r_bc[:], ALU.mult)

        ps_out = ps_sm.tile([BC, IN_DIM], F32, tag="ps_small", name="ps_out")
        nc.tensor.matmul(ps_out[:], ones_r[0:1, 0:BC],
                         cb2[0:1, B2_BRO:B2_BRO + IN_DIM],
                         start=True, stop=False)
        nc.tensor.matmul(ps_out[:], ro3[:],
                         cb[:, C_WRO:C_WRO + IN_DIM],
                         start=False, stop=True)
        out_sb = sm.tile([BC, IN_DIM], F32, tag="outsb", name="out_sb")
        nc.vector.tensor_copy(out_sb[:], ps_out[:])
        nc.sync.dma_start(out, out_sb[:])


_NC_CACHE = None


def _get_nc():
    global _NC_CACHE
    if _NC_CACHE is None:
        _NC_CACHE = build_nc()
    return _NC_CACHE


def make_blob(inputs, core):
    blob = np.zeros((128, C_TOT), dtype=np.float32)
    W = {k: np.ascontiguousarray(inputs[k], dtype=np.float32)
         for k in ("W_write", "W_wq", "W_rq", "W_ro")}
    for cw, k in ((C_WWR, "W_write"), (C_WWQ, "W_wq"), (C_WRQ, "W_rq")):
        blob[:, cw:cw + 128] = W[k][0:128, :]
        blob[:, cw + 128:cw + 256] = W[k][128:256, :]
    blob[:, C_WRO:C_WRO + IN_DIM] = W["W_ro"]
    xt = np.ascontiguousarray(
        np.asarray(inputs["x"], dtype=np.float32)[core * BC:(core + 1) * BC]).T
    blob[:, C_XT:C_XT + BC] = xt[0:128, :]
    blob[:, C_XT + BC:C_XT + 2 * BC] = xt[128:256, :]
    blob[:, C_ONES:C_ONES + 128] = 1.0
    blob[:, C_IDENT:C_IDENT + 128] = np.eye(128, dtype=np.float32)
    blob2 = np.zeros((1, B2_TOT), dtype=np.float32)
    blob2[0, B2_BWR:B2_BWR + D] = np.asarray(inputs["b_write"], dtype=np.float32)
    blob2[0, B2_BWQ:B2_BWQ + D] = np.asarray(inputs["b_wq"], dtype=np.float32)
    blob2[0, B2_BRQ:B2_BRQ + D] = np.asarray(inputs["b_rq"], dtype=np.float32)
    blob2[0, B2_BRO:B2_BRO + IN_DIM] = np.asarray(inputs["b_ro"], dtype=np.float32)
    return blob, blob2


def make_in_maps(inputs):
    memt = np.ascontiguousarray(
        np.asarray(inputs["memory"], dtype=np.float32).transpose(0, 2, 1))
    in_maps = []
    for i in range(N_CORES):
        b1, b2 = make_blob(inputs, i)
        m = {"blob": b1, "blob2": b2,
             "memoryT": np.ascontiguousarray(memt[i * BC:(i + 1) * BC])}
        in_maps.append(m)
    return in_maps


def kernel(**inputs) -> np.ndarray:
    nc = _get_nc()
    in_maps = make_in_maps(inputs)
    res = run_bass_kernel_spmd(nc, in_maps, list(range(N_CORES)))
    out = np.concatenate([res.results[i]["out"] for i in range(N_CORES)], axis=0)
    return np.ascontiguousarray(out, dtype=np.float32)


if __name__ == "__main__":
    nc = build_nc()
    print("built ok; instructions:",
          sum(len(bb.instructions) for bb in nc.main_func.blocks))


# revision 56
# speedup vs baseline: 1.8256x; 1.0023x over previous
"""AttentionMemoryInterface Trainium2 kernel (v6).

Reference computation per batch element b (memory [N=4096, D=128], x [256]):
    mv = x@W_write+b_write; wq = x@W_wq+b_wq; rq = x@W_rq+b_rq
    wl[n] = mem[n,:]@wq ; ww = softmax(wl)
    new_mem = mem*(1-ww) + mv*ww
    rl[n] = new_mem[n,:]@rq ; rw = softmax(rl)
    out = (rw @ new_mem) @ W_ro + b_ro

Algebraic restructure (new_mem never materialized):
    lr[n] = mem[n,:]@rq                  (same pass as wl)
    cbar  = mv@rq                        (scalar per b)
    rl[n] = lr[n] + ww[n]*(cbar - lr[n])
    g[n]  = rw[n]*(1-ww[n]);  s = sum_n rw[n]*ww[n]
    read_out = sum_n g[n]*mem[n,:] + s*mv
    out = read_out @ W_ro + b_ro

v6 structure (per core, 8 batch elements, data-parallel over batch):
  - all small constants (weights, biases, pre-transposed x, ones, ident)
    packed host-side into one [128,1936] blob -> a single DMA, killing
    the per-DMA HWDGE serialization that stalled startup by ~10us.
  - memT [128(d), 4096(n)] tiles DMA'd in 8 chunks each so small DMAs
    (g-row folds) can slot into the serialized DMA-engine queue.
  - PE p-state warm-up burst of dummy matmuls while the first memory
    tiles stream in (the cost model's tensor engine clocks up only
    after sustained work).
  - pass 1: per 128-wide n-block, one matmul with STATIONARY = memT
    block [128(d),128(n)] and MOVING = (wq_b|rq_b) [128,2] -> PSUM
    [128(n_sub), 64] directly in softmax-friendly layout (col 2k = wl
    block k, col 2k+1 = lr block k). Output free size is 2, so pass-1
    PE time is negligible and no logit fix-up transposes are needed.
  - softmax: logits staged to SBUF once per b, then v3-proven ops only:
    ACT exp with accum_out, PE ones-matmul column sums + broadcasts,
    DVE reciprocal / stt / tsm. The read-softmax normalization (1/S2)
    is deferred and applied once in the epilogue.
  - g: PE transpose -> ACT copy -> gpsimd SWDGE fold (casting f32 ->
    f32r) -> g_row [1,4096] in n order, issued with high_priority so
    the tile scheduler doesn't starve it behind the memT chunk DMAs.
  - pass 2: PE broadcasts of g_row (f32r moving, 1 cycle/row) fill
    [128,1024] two-bank PSUM pairs; DVE scalar_tensor_tensor
    (memT * bcast) with accum_out, one op per pair (halves the
    per-instruction PSUM-access overhead); one column-reduce per b.
"""

import numpy as np

import concourse.bass as bass
import concourse.bass_isa as bass_isa
import concourse.bacc as bacc
import concourse.mybir as mybir
import concourse.tile as tile
from concourse.bass_utils import run_bass_kernel_spmd

N_CORES = 8
B, IN_DIM, D, N_SLOTS = 64, 256, 128, 4096
BC = B // N_CORES          # batch per core
NBLK = N_SLOTS // 128      # 32 n-blocks per b
NGRP = N_SLOTS // 512      # 8 pass-2 quarters per b
F32 = mybir.dt.float32
F32R = mybir.dt.float32r
BF16 = mybir.dt.bfloat16
AX = mybir.AxisListType
ALU = mybir.AluOpType
ACTF = mybir.ActivationFunctionType
RED = bass_isa.ReduceOp

# const blob column offsets
C_WWR = 0            # 2 x [128,128]
C_WWQ = 256
C_WRQ = 512
C_WRO = 768          # [128,256]
C_XT = 1024          # 2 x [128,8]
C_ONES = 1040        # [128,1] ones column
C_IDENT = 1041       # [128,128] identity (PE transpose)
C_TOT = 1169
# row-blob (biases live on partition 0 only; shipping them in the [128,*]
# blob wasted 0.33MB of serialized DMA ahead of the memory tiles)
B2_BWR = 0           # [1,128] each
B2_BWQ = 128
B2_BRQ = 256
B2_BRO = 384         # [1,256]
B2_ONES = 640        # [1,128] ones row
B2_TOT = 768


def build_nc(loop_n: int = 1, phase: str = "full", dve_q: int = 5,
             n_warm: int = 6, n_fill: int = 0):
    nc = bacc.Bacc("TRN2", target_bir_lowering=False, debug=False,
                   num_devices=N_CORES)

    blob_d = nc.dram_tensor("blob", [128, C_TOT], F32, kind="ExternalInput")
    blob2_d = nc.dram_tensor("blob2", [1, B2_TOT], F32, kind="ExternalInput")
    memt_d = nc.dram_tensor("memoryT", [BC, D, N_SLOTS], F32,
                            kind="ExternalInput")
    out_d = nc.dram_tensor("out", [BC, IN_DIM], F32, kind="ExternalOutput")

    kw = dict(blob=blob_d.ap(), blob2=blob2_d.ap(), memt=memt_d.ap(),
              out=out_d.ap(),
              phase=phase, dve_q=dve_q, n_warm=n_warm, n_fill=n_fill)
    with tile.TileContext(nc) as tc:
        if loop_n == 1:
            _body(nc, tc, **kw)
        else:
            with tc.For_i(0, loop_n, 1):
                _body(nc, tc, **kw)
    nc.compile()
    return nc


def _body(nc, tc, *, blob, blob2, memt, out, phase, dve_q, n_warm, n_fill):
    from contextlib import ExitStack
    ctx = ExitStack()
    with ctx:
        consts = ctx.enter_context(tc.tile_pool(name="consts", bufs=1))
        mtp = ctx.enter_context(tc.tile_pool(name="mt", bufs=1))
        sm = ctx.enter_context(tc.tile_pool(name="sm", bufs=2))
        grp = ctx.enter_context(tc.tile_pool(name="gr", bufs=3))
        trs = ctx.enter_context(tc.tile_pool(name="trs", bufs=2))
        ps_lt = ctx.enter_context(tc.tile_pool(name="ps_lt", bufs=2, space="PSUM"))
        ps_gb2 = ctx.enter_context(tc.tile_pool(name="ps_gb2", bufs=2, space="PSUM"))
        ps_sm = ctx.enter_context(tc.tile_pool(name="ps_sm", bufs=1, space="PSUM"))

        # ---------- const blob (one DMA) ----------
        cb = consts.tile([128, C_TOT], F32, tag="blob", name="cb")
        nc.sync.dma_start(cb[:], blob)
        # f32r copy of the ones row (DMA cast rounds; bitcast is rejected by
        # the BIR verifier for f32r matmul operands)
        ones_rr = consts.tile([1, 128], F32R, tag="onesrr", name="ones_rr")
        nc.gpsimd.dma_start(ones_rr[:], blob2[0:1, B2_ONES:B2_ONES + 128])
        # bias/ones row blob on the ACT ring (off the SP/memory DMA window)
        cb2 = consts.tile([1, B2_TOT], F32, tag="blob2", name="cb2")
        nc.scalar.dma_start(cb2[:], blob2)
        ones_r = cb2[0:1, B2_ONES:B2_ONES + 128]

        # ---------- memory DMAs (flat, per-partition contiguous) ----------
        # chunked so small DMAs (g_row folds, out) can slot into the
        # serialized DMA-engine queue between transfers
        m_tiles = []
        MCH = 8
        assert N_SLOTS % MCH == 0 or True
        for b in range(BC):
            mb = mtp.tile([128, N_SLOTS], F32, tag=f"mem{b}", name=f"memt{b}")
            bounds = [N_SLOTS * c // MCH for c in range(MCH + 1)]
            for c in range(MCH):
                nc.sync.dma_start(mb[:, bounds[c]:bounds[c + 1]],
                                  memt[b][:, bounds[c]:bounds[c + 1]])
            m_tiles.append(mb)

        if phase == "dma":
            dummy = sm.tile([128, BC], F32, tag="dummy", name="dummy")
            for b in range(BC):
                nc.vector.tensor_copy(dummy[:, b:b + 1], m_tiles[b][:, 0:1])
            return

        # ---------- PE p-state warm-up (dummy f32 matmuls) ----------
        for i in range(n_warm):
            psw = ps_gb2.tile([128, 1024], F32, tag="ps_gb2", name=f"warm{i}")
            nc.tensor.matmul(psw[:, 0:128], ones_r,
                             cb[0:1, C_WWQ:C_WWQ + 128])

        # ---------- projections -> mv_t [128, BC], qp [128, 2*BC] ----------
        # qp columns: col 2b = wq_b, col 2b+1 = rq_b.
        mv_t = consts.tile([128, BC], F32, tag="mvt", name="mv_t")
        qp = consts.tile([128, 2 * BC], F32, tag="qp", name="qp")
        for j, (cw, cbias) in enumerate(((C_WWR, B2_BWR), (C_WWQ, B2_BWQ),
                                         (C_WRQ, B2_BRQ))):
            ps = ps_sm.tile([128, BC], F32, tag="ps_small", name=f"ps_proj{j}")
            nc.tensor.matmul(ps[:], cb2[0:1, cbias:cbias + D],
                             ones_r[0:1, 0:BC], start=True, stop=False)
            for k in range(IN_DIM // 128):
                nc.tensor.matmul(
                    ps[:], cb[:, cw + k * 128:cw + (k + 1) * 128],
                    cb[:, C_XT + k * BC:C_XT + (k + 1) * BC],
                    start=False, stop=(k == IN_DIM // 128 - 1))
            if j == 0:
                nc.scalar.activation(mv_t[:], ps[:], ACTF.Copy)
            else:
                nc.scalar.activation(qp[:, (j - 1)::2], ps[:], ACTF.Copy)

        # ---------- cbar (mv . rq per b) ----------
        ones_c = cb[:, C_ONES:C_ONES + 1]
        tmv = sm.tile([128, BC], F32, tag="tmv", name="tmv")
        nc.vector.tensor_tensor(tmv[:], mv_t[:], qp[:, 1::2], ALU.mult)
        ps_c = ps_sm.tile([1, BC], F32, tag="ps_small", name="ps_crow")
        nc.tensor.matmul(ps_c[:], ones_c, tmv[:])
        c_row = consts.tile([1, BC], F32, tag="crow", name="c_row")
        nc.scalar.activation(c_row[:], ps_c[:], ACTF.Copy)
        ps_cb = ps_sm.tile([128, BC], F32, tag="ps_small", name="ps_cbc")
        nc.tensor.matmul(ps_cb[:], ones_r, c_row[:])
        c_bc = consts.tile([128, BC], F32, tag="cbc", name="c_bc")
        nc.scalar.activation(c_bc[:], ps_cb[:], ACTF.Copy)

        # accumulators / per-b scalar rows (bcast once in the epilogue)
        ro_t = sm.tile([128, BC], F32, tag="rot", name="ro_t", bufs=1)
        ps_rows = ps_sm.tile([1, 2 * BC], F32, tag="ps_rows", name="ps_rows",
                             bufs=1)
        ps_srow = ps_rows[0:1, 0:BC]
        ps_s2row = ps_rows[0:1, BC:2 * BC]

        lt_tiles = [None] * BC
        g_state = [None] * BC

        def stage1(b):
            # logits: stationary = memT n-block (f32r), moving = (wq_b|rq_b)
            mb = m_tiles[b]
            ps = ps_lt.tile([128, 2 * NBLK], F32, tag="ps_lt", name=f"pslt{b}")
            lt_tiles[b] = ps
            q2 = qp[:, 2 * b:2 * b + 2]
            for k in range(NBLK):
                nc.tensor.matmul(ps[:, 2 * k:2 * k + 2],
                                 mb[:, 128 * k:128 * (k + 1)],
                                 q2, start=True, stop=True)

        def stage2(b):
            # softmax chain -> unnormalized g (missing 1/S2), s' col
            ps = lt_tiles[b]
            wl = ps[:, 0::2]
            lr = ps[:, 1::2]

            e1 = sm.tile([128, NBLK], F32, tag="e1", name=f"e1_{b}")
            e1s = sm.tile([128, 1], F32, tag="e1s", name=f"e1s_{b}")
            nc.scalar.activation(e1[:], wl, ACTF.Exp, accum_out=e1s[:])
            s1a = sm.tile([128, 1], F32, tag="s1a", name=f"s1a_{b}")
            nc.gpsimd.partition_all_reduce(s1a[:], e1s[:], channels=128,
                                           reduce_op=RED.add)
            r1c = sm.tile([128, 1], F32, tag="r1c", name=f"r1c_{b}")
            nc.vector.reciprocal(r1c[:], s1a[:])

            # ww = e1/S1 ; t1 = (lr - cbar)*ww ; rl = lr - t1
            ww = sm.tile([128, NBLK], F32, tag="ww", name=f"ww_{b}")
            nc.vector.tensor_scalar_mul(ww[:], e1[:], r1c[:, 0:1])
            t1 = sm.tile([128, NBLK], F32, tag="t1", name=f"t1_{b}")
            nc.vector.scalar_tensor_tensor(
                t1[:], lr, c_bc[:, b:b + 1], ww[:],
                op0=ALU.subtract, op1=ALU.mult)
            rl = sm.tile([128, NBLK], F32, tag="rl", name=f"rl_{b}")
            nc.vector.scalar_tensor_tensor(
                rl[:], lr, 0.0, t1[:], op0=ALU.add, op1=ALU.subtract)

            e2 = sm.tile([128, NBLK], F32, tag="e2", name=f"e2_{b}")
            e2s = sm.tile([128, 1], F32, tag="e2s", name=f"e2s_{b}")
            nc.scalar.activation(e2[:], rl[:], ACTF.Exp,
                                 accum_out=e2s[:])
            s2a = sm.tile([128, 1], F32, tag="s2a", name=f"s2a_{b}")
            nc.gpsimd.partition_all_reduce(s2a[:], e2s[:], channels=128,
                                           reduce_op=RED.add)
            nc.vector.reciprocal(rcol[:, b:b + 1], s2a[:])

            # t2 = e2*ww ; g = e2 - t2. On Pool/ACT so the g -> fold path
            # never queues behind the previous batch's big DVE stt block.
            t2 = sm.tile([128, NBLK], F32, tag="t2", name=f"t2_{b}")
            nc.gpsimd.tensor_tensor(t2[:], e2[:], ww[:], ALU.mult)
            t2c = sm.tile([128, NBLK], F32, tag="t2c", name=f"t2c_{b}")
            t2s = sm.tile([128, 1], F32, tag="t2s", name=f"t2s_{b}")
            nc.scalar.activation(t2c[:], t2[:], ACTF.Copy, accum_out=t2s[:])
            g = sm.tile([128, NBLK], F32, tag="g", name=f"g_{b}")
            nc.gpsimd.tensor_tensor(g[:], e2[:], t2[:], ALU.subtract)
            s3a = sm.tile([128, 1], F32, tag="s3a", name=f"s3a_{b}")
            nc.gpsimd.partition_all_reduce(s3a[:], t2s[:], channels=128,
                                           reduce_op=RED.add)
            nc.vector.tensor_copy(scol[:, b:b + 1], s3a[:])

            # g [128(nsub), 32(k)] -> PE transpose -> gt [32(k), 128(nsub)]
            # (bf16, so the fold needs no cast and the g-broadcast matmuls
            # run at 1 cyc/row) -> two natural-AP half folds -> g_row
            # [1,4096] in n order (n = 128k + nsub).
            ps_gt = ps_sm.tile([NBLK, 128], F32, tag="ps_small",
                               name=f"ps_gt_{b}")
            nc.tensor.matmul(ps_gt[:], g[:],
                             cb[:, C_IDENT:C_IDENT + 128],
                             is_transpose=True)
            gt = sm.tile([NBLK, 128], F32, tag="gt", name=f"gt_{b}")
            nc.scalar.activation(gt[:], ps_gt[:], ACTF.Copy)
            g_row = grp.tile([1, N_SLOTS], F32R, tag="grow", name=f"grow_{b}",
                             bufs=3)
            # SWDGE (gpsimd) fold: casts f32 -> f32r while rearranging
            # partitions into one row (the proven v3 pattern)
            with tc.high_priority():
                nc.gpsimd.dma_start(g_row[0:1, :], gt[:])
            g_state[b] = g_row

        def stage3(b):
            # pass 2: weighted sum over n, all on DVE as 4 stt ops over
            # [128,1024] PSUM pairs (GPSIMD can't read PSUM / run stt, and
            # mixing Pool tensor ops with partition_all_reduce would thrash
            # Q7 ucode library reloads).
            mb = m_tiles[b]
            g_row = g_state[b]
            acc8 = sm.tile([128, 4], F32, tag="acc8", name=f"acc8_{b}")
            for p in range(4):
                psg2 = ps_gb2.tile([128, 1024], F32, tag="ps_gb2",
                                   name=f"psgb2_{b}_{p}")
                for h in range(2):
                    qi = 2 * p + h
                    nc.tensor.matmul(
                        psg2[:, h * 512:(h + 1) * 512], ones_rr[:],
                        g_row[0:1, qi * 512:(qi + 1) * 512])
                trash = trs.tile([128, 1024], F32, tag="trash0",
                                 name=f"trash_{b}_{p}")
                nc.vector.scalar_tensor_tensor(
                    trash[:], mb[:, p * 1024:(p + 1) * 1024], 1.0, psg2[:],
                    op0=ALU.mult, op1=ALU.mult, accum_out=acc8[:, p:p + 1])
            nc.vector.tensor_reduce(ro_t[:, b:b + 1], acc8[:], AX.X, ALU.add)

        if phase == "p1":
            for b in range(BC):
                stage1(b)
            return
        if phase == "sm":
            for t in range(BC + 1):
                if t < BC:
                    stage1(t)
                if t >= 1:
                    stage2(t - 1)
            return
        # issue order per slot: stage1(t) [PE], stage2(t) [ACT/DVE/Pool
        # chain], stage3(t-1) [PE bcasts + DVE/Pool stt]. stage2(t) before
        # stage3(t-1) keeps each engine's in-order stream aligned with data
        # readiness (chain ops for t become ready before t-1's stt).
        for t in range(BC + 1):
            if t < BC:
                stage1(t)
                stage2(t)
            if t >= 1:
                stage3(t - 1)

        # ---------- PE p-state keep-warm fillers ----------
        # No-dep worst-priority matmuls: the tile scheduler drips them into
        # every PE idle gap, keeping the p-state ramp from resetting between
        # real bursts. Issued last so all real work outranks them.
        if n_fill:
            psf = ps_sm.tile([128, 64], F32, tag="ps_fill", name="ps_fill",
                             bufs=1)
            for i in range(n_fill):
                nc.tensor.matmul(psf[:], ones_r,
                                 cb[0:1, C_WWR:C_WWR + 64])

        # ---------- epilogue ----------
        if phase != "full":
            return
        # ro3 = (ro_t + mv*s') / S2   (deferred read-softmax normalization)
        s_row = sm.tile([1, BC], F32, tag="s_row", name="s_row")
        nc.scalar.activation(s_row[:], ps_srow, ACTF.Copy)
        ps_sb = ps_sm.tile([128, BC], F32, tag="ps_small", name="ps_sb")
        nc.tensor.matmul(ps_sb[:], ones_r, s_row[:])
        s_bc = sm.tile([128, BC], F32, tag="s_bc", name="s_bc")
        nc.scalar.activation(s_bc[:], ps_sb[:], ACTF.Copy)
        s2_row = sm.tile([1, BC], F32, tag="s2_row", name="s2_row")
        nc.scalar.activation(s2_row[:], ps_s2row, ACTF.Copy)
        r2_row = sm.tile([1, BC], F32, tag="r2_row", name="r2_row")
        nc.vector.reciprocal(r2_row[:], s2_row[:])
        ps_rb = ps_sm.tile([128, BC], F32, tag="ps_small", name="ps_rb")
        nc.tensor.matmul(ps_rb[:], ones_r, r2_row[:])
        r_bc = sm.tile([128, BC], F32, tag="r_bc", name="r_bc")
        nc.scalar.activation(r_bc[:], ps_rb[:], ACTF.Copy)
        t3 = sm.tile([128, BC], F32, tag="t3", name="t3")
        nc.vector.tensor_tensor(t3[:], mv_t[:], s_bc[:], ALU.mult)
        ro2 = sm.tile([128, BC], F32, tag="ro2", name="ro2")
        nc.vector.tensor_tensor(ro2[:], ro_t[:], t3[:], ALU.add)
        ro3 = sm.tile([128, BC], F32, tag="ro3", name="ro3")
        nc.vector.tensor_tensor(ro3[:], ro2[:], r_bc[:], ALU.mult)

        ps_out = ps_sm.tile([BC, IN_DIM], F32, tag="ps_small", name="ps_out")
        nc.tensor.matmul(ps_out[:], ones_r[0:1, 0:BC],
                         cb2[0:1, B2_BRO:B2_BRO + IN_DIM],
                         start=True, stop=False)
        nc.tensor.matmul(ps_out[:], ro3[:],
                         cb[:, C_WRO:C_WRO + IN_DIM],
                         start=False, stop=True)
        out_sb = sm.tile([BC, IN_DIM], F32, tag="outsb", name="out_sb")
        nc.vector.tensor_copy(out_sb[:], ps_out[:])
        nc.sync.dma_start(out, out_sb[:])


_NC_CACHE = None


def _get_nc():
    global _NC_CACHE
    if _NC_CACHE is None:
        _NC_CACHE = build_nc()
    return _NC_CACHE


def make_blob(inputs, core):
    blob = np.zeros((128, C_TOT), dtype=np.float32)
    W = {k: np.ascontiguousarray(inputs[k], dtype=np.float32)
         for k in ("W_write", "W_wq", "W_rq", "W_ro")}
    for cw, k in ((C_WWR, "W_write"), (C_WWQ, "W_wq"), (C_WRQ, "W_rq")):
        blob[:, cw:cw + 128] = W[k][0:128, :]
        blob[:, cw + 128:cw + 256] = W[k][128:256, :]
    blob[:, C_WRO:C_WRO + IN_DIM] = W["W_ro"]
    xt = np.ascontiguousarray(
        np.asarray(inputs["x"], dtype=np.float32)[core * BC:(core + 1) * BC]).T
    blob[:, C_XT:C_XT + BC] = xt[0:128, :]
    blob[:, C_XT + BC:C_XT + 2 * BC] = xt[128:256, :]
    blob[:, C_ONES:C_ONES + 1] = 1.0
    blob[:, C_IDENT:C_IDENT + 128] = np.eye(128, dtype=np.float32)
    blob2 = np.zeros((1, B2_TOT), dtype=np.float32)
    blob2[0, B2_BWR:B2_BWR + D] = np.asarray(inputs["b_write"], dtype=np.float32)
    blob2[0, B2_BWQ:B2_BWQ + D] = np.asarray(inputs["b_wq"], dtype=np.float32)
    blob2[0, B2_BRQ:B2_BRQ + D] = np.asarray(inputs["b_rq"], dtype=np.float32)
    blob2[0, B2_BRO:B2_BRO + IN_DIM] = np.asarray(inputs["b_ro"], dtype=np.float32)
    blob2[0, B2_ONES:B2_ONES + 128] = 1.0
    return blob, blob2


def make_in_maps(inputs):
    memt = np.ascontiguousarray(
        np.asarray(inputs["memory"], dtype=np.float32).transpose(0, 2, 1))
    in_maps = []
    for i in range(N_CORES):
        b1, b2 = make_blob(inputs, i)
        m = {"blob": b1, "blob2": b2,
             "memoryT": np.ascontiguousarray(memt[i * BC:(i + 1) * BC])}
        in_maps.append(m)
    return in_maps


def kernel(**inputs) -> np.ndarray:
    nc = _get_nc()
    in_maps = make_in_maps(inputs)
    res = run_bass_kernel_spmd(nc, in_maps, list(range(N_CORES)))
    out = np.concatenate([res.results[i]["out"] for i in range(N_CORES)], axis=0)
    return np.ascontiguousarray(out, dtype=np.float32)


if __name__ == "__main__":
    nc = build_nc()
    print("built ok; instructions:",
          sum(len(bb.instructions) for bb in nc.main_func.blocks))
